# revision 7
# baseline (speedup 1.0000x reference)
"""Trainium2 Bass kernel for InverseImportanceLinear.

out = x @ W_deq.T + bias, where
  W_deq[k,n] = (Q[k,n] - zeros[k, n//64]) * scales[k, n//64] * mu2[k] * mu1[n]

Sharding: tensor-parallel over K (output features) across 8 cores.
x and mu1 replicated; Q/scales/zeros/mu2/bias sharded along K.
Each core computes out[:, k_shard]; host concatenates along K.

Per-core pipeline (all compute on device):
  W path: DMA Q (int32, natural [k,n] layout) -> fused (Q - z) * (s*mu2)
          dequant into fp16 via per-group tensor_scalar (DVE) / activation
          (ACT), -> PE transpose 128x128 blocks -> PSUM->SBUF copy fused
          with a per-partition mu1 multiply -> W.T resident in SBUF as
          [128, N/128, K_shard] fp16.
  x path: DMA x fp32 -> convert fp16 -> bounce via DRAM ->
          dma_start_transpose -> x.T tiles [128, N/128, 128] fp16.
  main:   for each 128-token tile: 3 psum tiles (k-blocks 512/512/384),
          accumulate matmuls over the 32 n-chunks, plus one ones-row
          matmul that folds in bias; copy psum -> sbuf fp32; DMA out.
"""

from contextlib import ExitStack

import numpy as np

import concourse.bass as bass
import concourse.mybir as mybir
import concourse.tile as tile
from concourse import bacc
from concourse.bass_utils import run_bass_kernel_spmd
from concourse.masks import make_identity

FP16 = mybir.dt.float16
FP32 = mybir.dt.float32
INT32 = mybir.dt.int32

N_CORES = 8

# Full-problem dims (hardcoded per contract; kernel.py must be self-contained).
T_FULL, N_FULL, K_FULL, GS_FULL = 4096, 4096, 11264, 64


def ceil_div(a, b):
    return (a + b - 1) // b


def build_program(T, N, KS, GS, num_devices=N_CORES):
    """Build the per-core SPMD program.

    T: tokens, N: contraction dim, KS: per-core output features,
    GS: quant group size along N.
    """
    P = 128
    TT = T // P          # token tiles
    PO = N // P          # n-chunks
    KO = KS // P         # k-tiles of the shard
    NGRP = N // GS       # groups per k-row
    GPC = P // GS if GS < P else 1  # groups per 128-n-chunk (full cfg: 2)
    assert T % P == 0 and N % P == 0 and KS % P == 0 and N % GS == 0

    KB = 512             # k-block width (psum free dim)
    k_blocks = []
    k0 = 0
    while k0 < KS:
        k_blocks.append((k0, min(KB, KS - k0)))
        k0 += KB

    # stage width for Q / x staging tiles (bytes/partition kept modest)
    SW = min(N, 2048)
    NSW = N // SW

    nc = bacc.Bacc(
        "TRN2", target_bir_lowering=False, debug=False, num_devices=num_devices
    )

    x_d = nc.dram_tensor("x", [T, N], FP32, kind="ExternalInput")
    q_d = nc.dram_tensor("q", [KS, N], INT32, kind="ExternalInput")
    scales_d = nc.dram_tensor("scales", [KS, NGRP], FP32, kind="ExternalInput")
    zeros_d = nc.dram_tensor("zeros", [KS, NGRP], FP32, kind="ExternalInput")
    mu1_d = nc.dram_tensor("mu1", [N], FP32, kind="ExternalInput")
    mu2_d = nc.dram_tensor("mu2", [KS], FP32, kind="ExternalInput")
    bias_d = nc.dram_tensor("bias", [KS], FP32, kind="ExternalInput")
    out_d = nc.dram_tensor("out", [T, KS], FP32, kind="ExternalOutput")

    # rearranged DRAM views
    q_r = q_d.ap().rearrange("(ko p) n -> p ko n", p=P)           # [128, KO, N]
    sc_r = scales_d.ap().rearrange("(ko p) g -> p ko g", p=P)     # [128, KO, NGRP]
    zr_r = zeros_d.ap().rearrange("(ko p) g -> p ko g", p=P)      # [128, KO, NGRP]
    mu2_r = mu2_d.ap().rearrange("(ko p) -> p ko", p=P)           # [128, KO]
    mu1_r = mu1_d.ap().rearrange("(po p) -> p po", p=P)           # [128, PO]

    with tile.TileContext(nc) as tc, ExitStack() as ctx:
        consts = ctx.enter_context(tc.tile_pool(name="consts", bufs=1))
        dram = ctx.enter_context(tc.tile_pool(name="dram", bufs=1, space="DRAM"))
        qpool = ctx.enter_context(tc.tile_pool(name="qpool", bufs=2))
        wpool = ctx.enter_context(tc.tile_pool(name="wpool", bufs=2))
        xpool = ctx.enter_context(tc.tile_pool(name="xpool", bufs=2))
        x16pool = ctx.enter_context(tc.tile_pool(name="x16pool", bufs=2))
        xtpool = ctx.enter_context(tc.tile_pool(name="xtpool", bufs=2))
        smallp = ctx.enter_context(tc.tile_pool(name="smallp", bufs=2))
        outp = ctx.enter_context(tc.tile_pool(name="outp", bufs=4))
        wres = ctx.enter_context(tc.tile_pool(name="wres", bufs=1))
        psum_t = ctx.enter_context(tc.tile_pool(name="psum_t", bufs=2, space="PSUM"))
        psum_m = ctx.enter_context(tc.tile_pool(name="psum_m", bufs=4, space="PSUM"))

        # ---- constants ----
        ident = consts.tile([P, P], FP16)
        make_identity(nc, ident)

        mu1t = consts.tile([P, PO], FP32)
        nc.sync.dma_start(mu1t[:], mu1_r)

        mu2t = consts.tile([P, KO], FP32)
        nc.sync.dma_start(mu2t[:], mu2_r)

        sct = consts.tile([P, KO, NGRP], FP32)
        nc.sync.dma_start(sct[:], sc_r)
        zrt = consts.tile([P, KO, NGRP], FP32)
        nc.sync.dma_start(zrt[:], zr_r)

        ones16 = consts.tile([1, P], FP16)
        nc.vector.memset(ones16[:], 1.0)

        bias_f32 = consts.tile([1, KS], FP32)
        nc.sync.dma_start(bias_f32[:], bias_d.ap()[None, :])
        bias16 = consts.tile([1, KS], FP16)
        nc.vector.tensor_copy(bias16[:], bias_f32[:])

        # W.T resident: [128 (n within chunk), PO, KS] fp16
        wt = wres.tile([P, PO, KS], FP16)

        # x16 bounce in DRAM
        x16_d = dram.tile([T, N], FP16)
        x16_r = x16_d.rearrange("t (po p) -> t po p", p=P)  # [T, PO, 128]

        # ---- W path: dequant + PE transpose, per k-tile ----
        gs_per_stage = SW // GS
        for ko in range(KO):
            # per-k-tile group coefficients
            smu = smallp.tile([P, NGRP], FP32, tag="smu")
            nc.vector.tensor_scalar_mul(smu[:], sct[:, ko, :], mu2t[:, ko : ko + 1])
            for sw in range(NSW):
                qs = qpool.tile([P, SW], INT32)
                nc.sync.dma_start(qs[:], q_r[:, ko, sw * SW : (sw + 1) * SW])
                w16 = wpool.tile([P, SW], FP16)
                for g in range(gs_per_stage):
                    gg = sw * gs_per_stage + g  # global group idx in row
                    cols = slice(g * GS, (g + 1) * GS)
                    nc.vector.tensor_scalar(
                        w16[:, cols],
                        qs[:, cols],
                        zrt[:, ko, gg : gg + 1],
                        smu[:, gg : gg + 1],
                        mybir.AluOpType.subtract,
                        mybir.AluOpType.mult,
                    )
                # PE-transpose each 128x128 block of w16 into psum, then
                # copy to resident W.T with fused mu1 scale.
                po_base = sw * (SW // P)
                for pb in range(0, SW // P, 4):
                    nblk = min(4, SW // P - pb)
                    pt = psum_t.tile([P, 4 * P], FP16, tag="tpsum")
                    for j in range(nblk):
                        nc.tensor.transpose(
                            pt[:, j * P : (j + 1) * P],
                            w16[:, (pb + j) * P : (pb + j + 1) * P],
                            ident[:],
                        )
                    for j in range(nblk):
                        po = po_base + pb + j
                        nc.scalar.activation(
                            wt[:, po, ko * P : (ko + 1) * P],
                            pt[:, j * P : (j + 1) * P],
                            mybir.ActivationFunctionType.Copy,
                            scale=mu1t[:, po : po + 1],
                        )

        # ---- x path: convert fp32 -> fp16, bounce via DRAM ----
        for tt in range(TT):
            t0 = tt * P
            for sw in range(NSW):
                xs = xpool.tile([P, SW], FP32)
                nc.sync.dma_start(
                    xs[:], x_d.ap()[t0 : t0 + P, sw * SW : (sw + 1) * SW]
                )
                x16s = x16pool.tile([P, SW], FP16)
                nc.any.tensor_copy(x16s[:], xs[:])
                nc.sync.dma_start(
                    x16_d[t0 : t0 + P, sw * SW : (sw + 1) * SW], x16s[:]
                )

        # ---- main loop ----
        for tt in range(TT):
            t0 = tt * P
            xt = xtpool.tile([P, PO, P], FP16)
            nc.sync.dma_start_transpose(xt[:], x16_r[t0 : t0 + P])
            for (k0, kw) in k_blocks:
                ps_full = psum_m.tile([P, KB], FP32, tag="mpsum", name="mpsum")
                ps = ps_full[:, :kw]
                for po in range(PO):
                    nc.tensor.matmul(
                        ps,
                        xt[:, po, :],
                        wt[:, po, k0 : k0 + kw],
                        start=(po == 0),
                        stop=False,
                    )
                nc.tensor.matmul(
                    ps,
                    ones16[:, :],
                    bias16[:, k0 : k0 + kw],
                    start=False,
                    stop=True,
                )
                ob_full = outp.tile([P, KB], FP32, tag="ob", name="ob")
                ob = ob_full[:, :kw]
                nc.any.tensor_copy(ob, ps)
                nc.sync.dma_start(out_d.ap()[t0 : t0 + P, k0 : k0 + kw], ob)

    nc.compile()
    return nc


_CACHED = {}


def _get_program(key):
    if key not in _CACHED:
        T, N, KS, GS = key
        _CACHED[key] = build_program(T, N, KS, GS)
    return _CACHED[key]


def kernel(x, Q, scales, zeros, mu1, mu2, bias):
    """Full-input entry point. Shards K across 8 cores, runs SPMD, gathers."""
    T, N = x.shape
    K = Q.shape[0]
    GS = N // scales.shape[1]
    assert K % N_CORES == 0
    KS = K // N_CORES

    nc = _get_program((T, N, KS, GS))

    x = np.ascontiguousarray(x, dtype=np.float32)
    in_maps = []
    for c in range(N_CORES):
        ks = slice(c * KS, (c + 1) * KS)
        in_maps.append(
            {
                "x": x,
                "q": np.ascontiguousarray(Q[ks], dtype=np.int32),
                "scales": np.ascontiguousarray(scales[ks], dtype=np.float32),
                "zeros": np.ascontiguousarray(zeros[ks], dtype=np.float32),
                "mu1": np.ascontiguousarray(mu1, dtype=np.float32),
                "mu2": np.ascontiguousarray(mu2[ks], dtype=np.float32),
                "bias": np.ascontiguousarray(bias[ks], dtype=np.float32),
            }
        )

    res = run_bass_kernel_spmd(nc, in_maps, core_ids=list(range(N_CORES)))
    return np.concatenate([res.results[c]["out"] for c in range(N_CORES)], axis=1)
